# revision 24
# baseline (speedup 1.0000x reference)
"""Correlation-loss kernel for Trainium2 (8 NeuronCores, SPMD data-parallel).

Problem: for 800 random 16x16 patches of a 64-channel MSI image (first 32
channels used) and a 3-channel HE image, compute per-patch masked pairwise
squared-distance matrices over the 256 positions for both modalities and
L1-compare them; output sum(per-patch mean)/160.

Formulation: per patch, with mask m and sqm = (sum_c msi^2 - sum_c he^2)m/2,
    out[a,b] = -(dm-dh)[a,b]/2 * m[a]m[b]
is a single rank-39 matmul lhsT.T @ rhs with (hi+lo = sqm split so each part
fits fp8 precision; hi clipped to <=7.5 so the doubled copy stays in range)
    lhsT = [xm*m (32) | -xh*m (3) | -hi | -lo | -m | -m]  (K=39, cols=pos)
    rhs  = [xm*m (32) |  xh*m (3) |  m  |  m  | hi | lo]
and loss = sum_patches 2*sum|out| / 256^2 / 160 (abs kills the global sign).
out is symmetric, so only three 128x128 blocks are computed per patch: D1
(upper diagonal), B (off-diagonal, weight 2), D2 (lower diagonal). The x2
weight of B is baked in on the host by doubling rhs columns 128:256 (exact
in fp8); the D2 matmul reuses those doubled columns with its lhsT half
pre-scaled by 0.5 (also exact), so only one rhs copy is shipped.

Operands ship as fp8_e3m4 (rel err ~1.2e-3 vs 2e-2 budget): mega is
3.28 MB/core, fully hidden behind the consumers.

PSUM packing: each patch yields three 128-col f32 granules (D1, 2B, D2).
Granules are packed densely, 4 per bank, 8 per 2-bank tile (2.67 patches
per tile), so the ACT/DVE consumers always stream a full 1024 cols per
instruction - per-op overhead (172-cycle PSUM ramp + 183 ns accumulator
read) is amortized over 33% more data than patch-aligned 384-col layouts.
Consumers alternate tiles ABAB (disjoint banks, engines run in parallel)
and do a plain abs-sum straight out of PSUM: ACT via in-place Abs +
accumulator (ScalarE writes PSUM faster than SBUF; the abs values are
discarded), DVE via abs tensor_reduce. Per-tile partial sums land in SBUF
slot arrays that are DMA'd out raw on two parallel rings; the host does
the final O(KB) reduction, so the output DMA (and its ~1.5 us HBM
completion receipt) starts the moment the last consumer op retires.

Granules are emitted in band-alternating order (even patch at partitions
0:39, odd at 64:103) so every LDWEIGHTS targets the opposite PE row group
from the in-flight MATMUL and prefetches instead of stalling. CRITICAL:
adjacent matmuls at different tile_positions execute concurrently in
different PE quadrants, so consecutive matmuls must target different PSUM
banks or the concurrent writes raise NRT_EXEC_UNIT_UNRECOVERABLE; granule
slot s therefore maps to (bank=s%2, pos=s//2). Multiple start=True
matmuls into disjoint regions of one bank are safe (has_written bits
clear bank-wide but data is preserved; verified on HW).

Memory layout: SBUF DMA bandwidth scales with the partition span of the
transfer, so operands are packed two-patches-per-128-partitions (64 is the
only legal matmul row offset for K=39) and DMAs span all 128 partitions
(anything else falls off the DIRECT2D fast path). Tile 0's two half-patch
chunks go down the scalar HWDGE ring concurrently with the sync ring's
bulk ladder: the gating first chunk completes earlier, and its transfer
doubles as the wake-up call for the 16th SDMA engine, which otherwise
starts ~3 us late and stalls the first matmul's completion semaphore.

Sharding: 100 patches per core, operands pre-gathered and quantized on
host, per-tile partial sums returned per core, final scalar on host.
"""

import os
import sys

sys.path.insert(0, "/opt/trn_rl_repo")

import ml_dtypes
import numpy as np

import concourse.bass as bass  # noqa: F401
import concourse.tile as tile
from concourse import bacc, mybir
from concourse.bass_utils import run_bass_kernel_spmd

WS = 16
NB = 800
TH = 0.05
P = WS * WS  # 256
HP = P // 2  # 128
N_CORES = 8
PPC = NB // N_CORES  # 100
HPC = PPC // 2  # 50 patches per band
K = 39
NGRAN = 3 * PPC  # 300 granules of 128 cols
GPT = 8  # granules per 2-bank PSUM tile
NT = (NGRAN + GPT - 1) // GPT  # 38 tiles (last holds 4 granules = 1 bank)
# input DMA chunk ladder (in half-patches; small first chunks let compute
# start while the bulk is still in flight).
DMA_LADDER = [1, 1, 3, 5, 7, 8, 8, 8, 9]

F32 = mybir.dt.float32
F8 = mybir.dt.float8e3
F8NP = ml_dtypes.float8_e3m4

LAST_EXEC_NS = None
LAST_RESULTS = None

_compiled = None

N_ACT_TILES = (NT + 1) // 2  # tiles 0,2,4,... -> ACT
N_DVE_TILES = NT // 2  # tiles 1,3,5,... -> DVE


def _granule(g):
    """granule index -> (h, parity, kind). Emission order per patch pair:
    D1e, D1o, Be, Bo, D2e, D2o — bands alternate so LDWEIGHTS prefetches."""
    pair, piece = divmod(g, 6)
    return pair, piece % 2, piece // 2


def _build_program():
    nc = bacc.Bacc(
        "TRN2", target_bir_lowering=False, debug=False, num_devices=N_CORES
    )

    # mega: per half-patch h, cols [0:256)=lhsT (cols 128:256 pre-halved),
    # [256:512)=rhs (cols 128:256 pre-doubled)
    # rows 0:39 even patches, 64:103 odd patches, zeros elsewhere
    mega_d = nc.dram_tensor("mega", [128, HPC, 2 * P], F8, kind="ExternalInput").ap()
    out_d = nc.dram_tensor(
        "partial", [128, N_ACT_TILES + N_DVE_TILES], F32, kind="ExternalOutput"
    ).ap()

    with tile.TileContext(nc) as tc:
        with (
            tc.tile_pool(name="ops", bufs=1) as opool,
            tc.tile_pool(name="psum", bufs=4, space="PSUM") as ppool,
            tc.tile_pool(name="accs", bufs=1) as apool,
        ):
            mega = opool.tile([128, HPC, 2 * P], F8)
            # tile 0's two half-patches go down the scalar HWDGE ring while
            # the sync ring issues the bulk in parallel: the first chunk's
            # completion (which gates everything) comes ~0.6us earlier, and
            # the scalar ring's first transfer doubles as the wake-up call
            # for the lazily-starting 16th SDMA engine.
            nc.scalar.dma_start(mega[:, 0:1], mega_d[:, 0:1])
            nc.scalar.dma_start(mega[:, 1:2], mega_d[:, 1:2])
            # bulk input on the sync ring in consumption order.
            off = 2
            for w in DMA_LADDER:
                sl = slice(off, off + w)
                off += w
                nc.sync.dma_start(mega[:, sl], mega_d[:, sl])

            # separate slot tiles per engine: a shared tile would thread a
            # WAW dependency between every ACT and DVE op
            slots_a = apool.tile([128, N_ACT_TILES], F32)
            slots_d = apool.tile([128, N_DVE_TILES], F32)
            zbias = apool.tile([128, 1], F32)
            nc.vector.memset(slots_a[:], 0.0)
            nc.vector.memset(slots_d[:], 0.0)
            nc.vector.memset(zbias[:], 0.0)

            for t in range(NT):
                g0, g1 = GPT * t, min(GPT * (t + 1), NGRAN)
                npos = (g1 - g0) // 2  # granules interleave across both banks
                ps = ppool.tile([128, 2, 2 * P], F32)
                for g in range(g0, g1):
                    h, parity, kind = _granule(g)
                    if parity == 0:
                        band, tp = slice(0, K), None
                    else:
                        band, tp = slice(64, 64 + K), (64, 0)
                    lc = slice(0, HP) if kind < 2 else slice(HP, P)
                    rc = slice(P, P + HP) if kind == 0 else slice(P + HP, 2 * P)
                    # adjacent matmuls run concurrently in different PE
                    # quadrants, so they MUST target different PSUM banks:
                    # bank = slot parity, pos fills 0..3 within the bank.
                    slot = g - g0
                    bank, pos = slot % 2, slot // 2
                    nc.tensor.matmul(
                        ps[:, bank, pos * HP : (pos + 1) * HP],
                        mega[band, h, lc],
                        mega[band, h, rc],
                        start=True,
                        stop=True,
                        tile_position=tp,
                    )

                # plain abs-sum of the tile, alternating engines ABAB
                if t % 2 == 0:
                    # in-place Abs: ScalarE writes PSUM faster than SBUF and
                    # the abs values are discarded anyway (only accum matters)
                    view = (
                        ps[:, :, :].rearrange("q a b -> q (a b)")
                        if npos == 4
                        else ps[:, :, 0 : npos * HP]
                    )
                    nc.scalar.activation(
                        view,
                        view,
                        mybir.ActivationFunctionType.Abs,
                        bias=zbias[:, 0:1],
                        accum_out=slots_a[:, t // 2 : t // 2 + 1],
                    )
                else:
                    nc.vector.tensor_reduce(
                        slots_d[:, t // 2 : t // 2 + 1],
                        ps[:, :, 0 : npos * HP],
                        axis=mybir.AxisListType.XY,
                        op=mybir.AluOpType.add,
                        apply_absolute_value=True,
                    )

            nc.scalar.dma_start(out_d[:, 0:N_ACT_TILES], slots_a[:])
            nc.sync.dma_start(out_d[:, N_ACT_TILES:], slots_d[:])

    nc.compile()
    return nc


def _prep_operands(tensor_msi, tensor_he, i_idx, j_idx):
    """Host gather + operand build.

    Returns mega [N_CORES,128,HPC,2P] fp8_e3m4.
    """
    msi = np.ascontiguousarray(tensor_msi[0, :32], dtype=np.float32)
    he = np.ascontiguousarray(tensor_he[0], dtype=np.float32)
    ii = np.asarray(i_idx).astype(np.int64)
    jj = np.asarray(j_idx).astype(np.int64)

    ig = np.broadcast_to((ii[:, None] + np.arange(WS))[:, :, None], (NB, WS, WS))
    jg = np.broadcast_to((jj[:, None] + np.arange(WS))[:, None, :], (NB, WS, WS))
    pm = msi[:, ig, jg].transpose(1, 0, 2, 3).reshape(NB, 32, P)
    ph = he[:, ig, jg].transpose(1, 0, 2, 3).reshape(NB, 3, P)

    # mask from the unquantized HE patch (matches the reference); everything
    # downstream is computed from the fp8-quantized values so the shipped
    # operands are self-consistent (diagonal of dm/dh stays ~0).
    m = (ph.sum(axis=1) >= TH).astype(np.float32)
    pmq = pm.astype(F8NP).astype(np.float32)
    phq = ph.astype(F8NP).astype(np.float32)
    sqm = 0.5 * ((pmq * pmq).sum(1) - (phq * phq).sum(1)) * m
    # hi/lo split of sqm: hi<=7.5 keeps the doubled copy (2*hi) below
    # e3m4's max of 15.5; lo is the remainder (tiny for sqm<7.5, up to a
    # few for larger sqm, so its ~3% quantization stays immaterial).
    hi = np.minimum(sqm.astype(F8NP).astype(np.float32), 7.5)
    lo = np.clip(sqm - hi, -7.75, 7.75)

    pm_m = pmq * m[:, None]
    ph_m = phq * m[:, None]

    c1 = lambda x: x[:, None]
    lhsT = np.concatenate(
        [pm_m, -ph_m, -c1(hi), -c1(lo), -c1(m), -c1(m)], axis=1
    )  # [NB, K, P]
    rhs = np.concatenate([pm_m, ph_m, c1(m), c1(m), c1(hi), c1(lo)], axis=1)
    # fold the x2 weight of the off-diagonal block into rhs cols 128:256 and
    # compensate the D2 matmul by pre-halving lhsT cols 128:256 (both exact).
    rhs[:, :, HP:] *= 2.0
    lhsT[:, :, HP:] *= 0.5

    lhsT = lhsT.reshape(N_CORES, PPC, K, P)
    rhs = rhs.reshape(N_CORES, PPC, K, P)

    mega = np.zeros((N_CORES, 128, HPC, 2 * P), dtype=F8NP)
    for par, base in ((0, 0), (1, 64)):
        rows = slice(base, base + K)
        # [N_CORES, HPC, K, P] -> [N_CORES, K, HPC, P]
        mega[:, rows, :, 0:P] = (
            lhsT[:, par::2].transpose(0, 2, 1, 3).astype(F8NP)
        )
        mega[:, rows, :, P : 2 * P] = (
            rhs[:, par::2].transpose(0, 2, 1, 3).astype(F8NP)
        )
    return np.ascontiguousarray(mega)


def kernel(tensor_msi, tensor_he, i_idx, j_idx, window_size, batch):
    global _compiled, LAST_EXEC_NS, LAST_RESULTS
    assert int(window_size) == WS and int(batch) == NB

    mega = _prep_operands(
        np.asarray(tensor_msi), np.asarray(tensor_he), i_idx, j_idx
    )

    if _compiled is None:
        _compiled = _build_program()
    nc = _compiled

    in_maps = [{"mega": mega[c]} for c in range(N_CORES)]

    trace = bool(os.environ.get("KERNEL_TRACE"))
    res = run_bass_kernel_spmd(
        nc, in_maps, core_ids=list(range(N_CORES)), trace=trace
    )
    LAST_EXEC_NS = res.exec_time_ns
    LAST_RESULTS = res

    # slot layout: [N_ACT_TILES | N_DVE_TILES], every slot used; the final
    # scalar reduction happens here on the host.
    total = np.float64(0.0)
    for c in range(N_CORES):
        total += res.results[c]["partial"].astype(np.float64).sum()
    loss = total * 2.0 / (P * P) / (NB // 5)
    return np.float32(loss)


# revision 25
# speedup vs baseline: 1.0226x; 1.0226x over previous
"""Correlation-loss kernel for Trainium2 (8 NeuronCores, SPMD data-parallel).

Problem: for 800 random 16x16 patches of a 64-channel MSI image (first 32
channels used) and a 3-channel HE image, compute per-patch masked pairwise
squared-distance matrices over the 256 positions for both modalities and
L1-compare them; output sum(per-patch mean)/160.

Formulation: per patch, with mask m and sqm = (sum_c msi^2 - sum_c he^2)m/2,
    out[a,b] = -(dm-dh)[a,b]/2 * m[a]m[b]
is a single rank-39 matmul lhsT.T @ rhs with (hi+lo = sqm split so each part
fits fp8 precision; hi clipped to <=7.5 so the doubled copy stays in range)
    lhsT = [xm*m (32) | -xh*m (3) | -hi | -lo | -m | -m]  (K=39, cols=pos)
    rhs  = [xm*m (32) |  xh*m (3) |  m  |  m  | hi | lo]
and loss = sum_patches 2*sum|out| / 256^2 / 160 (abs kills the global sign).
out is symmetric, so only three 128x128 blocks are computed per patch: D1
(upper diagonal), B (off-diagonal, weight 2), D2 (lower diagonal). The x2
weight of B is baked in on the host by doubling rhs columns 128:256 (exact
in fp8); the D2 matmul reuses those doubled columns with its lhsT half
pre-scaled by 0.5 (also exact), so only one rhs copy is shipped.

Operands ship as fp8_e3m4 (rel err ~1.2e-3 vs 2e-2 budget): mega is
3.28 MB/core, fully hidden behind the consumers.

PSUM packing: each patch yields three 128-col f32 granules (D1, 2B, D2).
Granules are packed densely, 4 per bank, 8 per 2-bank tile (2.67 patches
per tile), so the ACT/DVE consumers always stream a full 1024 cols per
instruction - per-op overhead (172-cycle PSUM ramp + 183 ns accumulator
read) is amortized over 33% more data than patch-aligned 384-col layouts.
Consumers alternate tiles ABAB (disjoint banks, engines run in parallel)
and do a plain abs-sum straight out of PSUM: ACT via in-place Abs +
accumulator (ScalarE writes PSUM faster than SBUF; the abs values are
discarded), DVE via abs tensor_reduce. Per-tile partial sums land in SBUF
slot arrays that are DMA'd out raw on two parallel rings; the host does
the final O(KB) reduction, so the output DMA (and its ~1.5 us HBM
completion receipt) starts the moment the last consumer op retires.

Granules are emitted in band-alternating order (even patch at partitions
0:39, odd at 64:103) so every LDWEIGHTS targets the opposite PE row group
from the in-flight MATMUL and prefetches instead of stalling. CRITICAL:
adjacent matmuls at different tile_positions execute concurrently in
different PE quadrants, so consecutive matmuls must target different PSUM
banks or the concurrent writes raise NRT_EXEC_UNIT_UNRECOVERABLE; granule
slot s therefore maps to (bank=s%2, pos=s//2). Multiple start=True
matmuls into disjoint regions of one bank are safe (has_written bits
clear bank-wide but data is preserved; verified on HW).

Memory layout: SBUF DMA bandwidth scales with the partition span of the
transfer, so operands are packed two-patches-per-128-partitions (64 is the
only legal matmul row offset for K=39) and DMAs span all 128 partitions
(anything else falls off the DIRECT2D fast path). Tile 0's two half-patch
chunks go down the scalar HWDGE ring concurrently with the sync ring's
bulk ladder: the gating first chunk completes earlier, and its transfer
doubles as the wake-up call for the 16th SDMA engine, which otherwise
starts ~3 us late and stalls the first matmul's completion semaphore.

Sharding: 100 patches per core, operands pre-gathered and quantized on
host, per-tile partial sums returned per core, final scalar on host.
"""

import os
import sys

sys.path.insert(0, "/opt/trn_rl_repo")

import ml_dtypes
import numpy as np

import concourse.bass as bass  # noqa: F401
import concourse.tile as tile
from concourse import bacc, mybir
from concourse.bass_utils import run_bass_kernel_spmd

WS = 16
NB = 800
TH = 0.05
P = WS * WS  # 256
HP = P // 2  # 128
N_CORES = 8
PPC = NB // N_CORES  # 100
HPC = PPC // 2  # 50 patches per band
K = 39
NGRAN = 3 * PPC  # 300 granules of 128 cols
GPT = 8  # granules per 2-bank PSUM tile
NT = (NGRAN + GPT - 1) // GPT  # 38 tiles (last holds 4 granules = 1 bank)
# input DMA chunk ladder (in half-patches; small first chunks let compute
# start while the bulk is still in flight).
DMA_LADDER = [1, 1, 3, 5, 7, 8, 8, 8, 9]

F32 = mybir.dt.float32
F8 = mybir.dt.float8e3
F8NP = ml_dtypes.float8_e3m4

LAST_EXEC_NS = None
LAST_RESULTS = None

_compiled = None

# tile 0 holds only 4 granules so the first consumer op is gated on a
# single input chunk and 4 matmuls; it goes to DVE (the slower engine gets
# the small op). Even tiles -> DVE, odd -> ACT.
N_ACT_TILES = NT // 2
N_DVE_TILES = (NT + 1) // 2


def _granule(g):
    """granule index -> (h, parity, kind). Emission order per patch pair:
    D1e, D1o, Be, Bo, D2e, D2o — bands alternate so LDWEIGHTS prefetches."""
    pair, piece = divmod(g, 6)
    return pair, piece % 2, piece // 2


def _build_program():
    nc = bacc.Bacc(
        "TRN2", target_bir_lowering=False, debug=False, num_devices=N_CORES
    )

    # mega: per half-patch h, cols [0:256)=lhsT (cols 128:256 pre-halved),
    # [256:512)=rhs (cols 128:256 pre-doubled)
    # rows 0:39 even patches, 64:103 odd patches, zeros elsewhere
    mega_d = nc.dram_tensor("mega", [128, HPC, 2 * P], F8, kind="ExternalInput").ap()
    out_d = nc.dram_tensor(
        "partial", [128, N_ACT_TILES + N_DVE_TILES], F32, kind="ExternalOutput"
    ).ap()

    with tile.TileContext(nc) as tc:
        with (
            tc.tile_pool(name="ops", bufs=1) as opool,
            tc.tile_pool(name="psum", bufs=4, space="PSUM") as ppool,
            tc.tile_pool(name="accs", bufs=1) as apool,
        ):
            mega = opool.tile([128, HPC, 2 * P], F8)
            # tile 0's two half-patches go down the scalar HWDGE ring while
            # the sync ring issues the bulk in parallel: the first chunk's
            # completion (which gates everything) comes ~0.6us earlier, and
            # the scalar ring's first transfer doubles as the wake-up call
            # for the lazily-starting 16th SDMA engine.
            nc.scalar.dma_start(mega[:, 0:1], mega_d[:, 0:1])
            # bulk input on the sync ring in consumption order.
            off = 1
            for w in DMA_LADDER:
                sl = slice(off, off + w)
                off += w
                nc.sync.dma_start(mega[:, sl], mega_d[:, sl])

            # separate slot tiles per engine: a shared tile would thread a
            # WAW dependency between every ACT and DVE op
            slots_a = apool.tile([128, N_ACT_TILES], F32)
            slots_d = apool.tile([128, N_DVE_TILES], F32)
            zbias = apool.tile([128, 1], F32)
            nc.vector.memset(slots_a[:], 0.0)
            nc.vector.memset(slots_d[:], 0.0)
            nc.vector.memset(zbias[:], 0.0)

            for t in range(NT):
                g0 = 0 if t == 0 else 4 + GPT * (t - 1)
                g1 = min(4 + GPT * t, NGRAN)
                npos = (g1 - g0) // 2  # granules interleave across both banks
                ps = ppool.tile([128, 2, 2 * P], F32)
                for g in range(g0, g1):
                    h, parity, kind = _granule(g)
                    if parity == 0:
                        band, tp = slice(0, K), None
                    else:
                        band, tp = slice(64, 64 + K), (64, 0)
                    lc = slice(0, HP) if kind < 2 else slice(HP, P)
                    rc = slice(P, P + HP) if kind == 0 else slice(P + HP, 2 * P)
                    # adjacent matmuls run concurrently in different PE
                    # quadrants, so they MUST target different PSUM banks:
                    # bank = slot parity, pos fills 0..3 within the bank.
                    slot = g - g0
                    bank, pos = slot % 2, slot // 2
                    nc.tensor.matmul(
                        ps[:, bank, pos * HP : (pos + 1) * HP],
                        mega[band, h, lc],
                        mega[band, h, rc],
                        start=True,
                        stop=True,
                        tile_position=tp,
                    )

                # plain abs-sum of the tile, alternating engines ABAB
                if t % 2 == 1:
                    # in-place Abs: ScalarE writes PSUM faster than SBUF and
                    # the abs values are discarded anyway (only accum matters)
                    view = (
                        ps[:, :, :].rearrange("q a b -> q (a b)")
                        if npos == 4
                        else ps[:, :, 0 : npos * HP]
                    )
                    nc.scalar.activation(
                        view,
                        view,
                        mybir.ActivationFunctionType.Abs,
                        bias=zbias[:, 0:1],
                        accum_out=slots_a[:, t // 2 : t // 2 + 1],
                    )
                else:
                    nc.vector.tensor_reduce(
                        slots_d[:, t // 2 : t // 2 + 1],
                        ps[:, :, 0 : npos * HP],
                        axis=mybir.AxisListType.XY,
                        op=mybir.AluOpType.add,
                        apply_absolute_value=True,
                    )

            nc.scalar.dma_start(out_d[:, 0:N_ACT_TILES], slots_a[:])
            nc.sync.dma_start(out_d[:, N_ACT_TILES:], slots_d[:])

    nc.compile()
    return nc


def _prep_operands(tensor_msi, tensor_he, i_idx, j_idx):
    """Host gather + operand build.

    Returns mega [N_CORES,128,HPC,2P] fp8_e3m4.
    """
    msi = np.ascontiguousarray(tensor_msi[0, :32], dtype=np.float32)
    he = np.ascontiguousarray(tensor_he[0], dtype=np.float32)
    ii = np.asarray(i_idx).astype(np.int64)
    jj = np.asarray(j_idx).astype(np.int64)

    ig = np.broadcast_to((ii[:, None] + np.arange(WS))[:, :, None], (NB, WS, WS))
    jg = np.broadcast_to((jj[:, None] + np.arange(WS))[:, None, :], (NB, WS, WS))
    pm = msi[:, ig, jg].transpose(1, 0, 2, 3).reshape(NB, 32, P)
    ph = he[:, ig, jg].transpose(1, 0, 2, 3).reshape(NB, 3, P)

    # mask from the unquantized HE patch (matches the reference); everything
    # downstream is computed from the fp8-quantized values so the shipped
    # operands are self-consistent (diagonal of dm/dh stays ~0).
    m = (ph.sum(axis=1) >= TH).astype(np.float32)
    pmq = pm.astype(F8NP).astype(np.float32)
    phq = ph.astype(F8NP).astype(np.float32)
    sqm = 0.5 * ((pmq * pmq).sum(1) - (phq * phq).sum(1)) * m
    # hi/lo split of sqm: hi<=7.5 keeps the doubled copy (2*hi) below
    # e3m4's max of 15.5; lo is the remainder (tiny for sqm<7.5, up to a
    # few for larger sqm, so its ~3% quantization stays immaterial).
    hi = np.minimum(sqm.astype(F8NP).astype(np.float32), 7.5)
    lo = np.clip(sqm - hi, -7.75, 7.75)

    pm_m = pmq * m[:, None]
    ph_m = phq * m[:, None]

    c1 = lambda x: x[:, None]
    lhsT = np.concatenate(
        [pm_m, -ph_m, -c1(hi), -c1(lo), -c1(m), -c1(m)], axis=1
    )  # [NB, K, P]
    rhs = np.concatenate([pm_m, ph_m, c1(m), c1(m), c1(hi), c1(lo)], axis=1)
    # fold the x2 weight of the off-diagonal block into rhs cols 128:256 and
    # compensate the D2 matmul by pre-halving lhsT cols 128:256 (both exact).
    rhs[:, :, HP:] *= 2.0
    lhsT[:, :, HP:] *= 0.5

    lhsT = lhsT.reshape(N_CORES, PPC, K, P)
    rhs = rhs.reshape(N_CORES, PPC, K, P)

    mega = np.zeros((N_CORES, 128, HPC, 2 * P), dtype=F8NP)
    for par, base in ((0, 0), (1, 64)):
        rows = slice(base, base + K)
        # [N_CORES, HPC, K, P] -> [N_CORES, K, HPC, P]
        mega[:, rows, :, 0:P] = (
            lhsT[:, par::2].transpose(0, 2, 1, 3).astype(F8NP)
        )
        mega[:, rows, :, P : 2 * P] = (
            rhs[:, par::2].transpose(0, 2, 1, 3).astype(F8NP)
        )
    return np.ascontiguousarray(mega)


def kernel(tensor_msi, tensor_he, i_idx, j_idx, window_size, batch):
    global _compiled, LAST_EXEC_NS, LAST_RESULTS
    assert int(window_size) == WS and int(batch) == NB

    mega = _prep_operands(
        np.asarray(tensor_msi), np.asarray(tensor_he), i_idx, j_idx
    )

    if _compiled is None:
        _compiled = _build_program()
    nc = _compiled

    in_maps = [{"mega": mega[c]} for c in range(N_CORES)]

    trace = bool(os.environ.get("KERNEL_TRACE"))
    res = run_bass_kernel_spmd(
        nc, in_maps, core_ids=list(range(N_CORES)), trace=trace
    )
    LAST_EXEC_NS = res.exec_time_ns
    LAST_RESULTS = res

    # slot layout: [N_ACT_TILES | N_DVE_TILES], every slot used; the final
    # scalar reduction happens here on the host.
    total = np.float64(0.0)
    for c in range(N_CORES):
        total += res.results[c]["partial"].astype(np.float64).sum()
    loss = total * 2.0 / (P * P) / (NB // 5)
    return np.float32(loss)
